# revision 9
# baseline (speedup 1.0000x reference)
"""Windowed (patch) attention kernel for 8 Trainium2 NeuronCores.

Problem: serialized point-cloud attention.
  qkv = feat @ Wqkv + bqkv ; qkv = qkv[order] -> windows of 256 rows
  per-window, per-head softmax attention ; out = attn_out[inverse] @ Wproj + bproj

Distribution strategy (per sharding hint): the permutation `order` is applied
host-side while sharding, so each core receives its 32 windows' rows already
gathered and channel-major (transposed).  All FLOPs (QKV proj, attention,
output proj) run on-device in bf16 with f32 PSUM accumulation.  `inverse`
scatter + bias adds are applied host-side (exact; row permutation commutes
with the row-wise projection, softmax is shift-invariant so the k-bias
cancels and the v-bias contributes bv @ Wproj to every row).
"""

import numpy as np
import ml_dtypes

import concourse.bass as bass
import concourse.mybir as mybir
from concourse import bacc
from concourse.tile import TileContext
from concourse.masks import make_identity
from concourse.bass_utils import run_bass_kernel_spmd

N = 65536
C = 512
H = 8
KW = 256          # window size
SCALE = 0.125
NCORES = 8
ROWS = N // NCORES        # 8192 rows per core
NWIN = ROWS // KW         # 32 windows per core
D = C // H                # 64 head dim

BF16 = mybir.dt.bfloat16
F32 = mybir.dt.float32


def build_nc():
    nc = bacc.Bacc("TRN2", target_bir_lowering=False, debug=False, num_devices=NCORES)

    xt = nc.declare_dram_parameter("xt", [C, ROWS], BF16, isOutput=False)
    wqkv = nc.declare_dram_parameter("wqkv", [C, 3 * C], BF16, isOutput=False)
    wproj = nc.declare_dram_parameter("wproj", [C, C], BF16, isOutput=False)
    out = nc.declare_dram_parameter("out", [ROWS, C], F32, isOutput=True)

    with TileContext(nc) as tc:
        with (
            tc.tile_pool(name="const", bufs=1) as const,
            tc.tile_pool(name="sb", bufs=2) as sb,
            tc.tile_pool(name="outp", bufs=3) as outp,
            tc.tile_pool(name="eh", bufs=3) as eh,
            tc.tile_pool(name="pqk", bufs=2, space="PSUM") as pqk,
            tc.tile_pool(name="ps", bufs=2, space="PSUM") as ps,
            tc.tile_pool(name="pot", bufs=2, space="PSUM") as pot,
            tc.tile_pool(name="pvf", bufs=2, space="PSUM") as pvf,
        ):
            # --- persistent tiles: weights + identity -----------------------
            wq_sb = const.tile([128, 4, 3 * C], BF16)
            wp_sb = const.tile([128, 4, C], BF16)
            nc.sync.dma_start(out=wq_sb, in_=wqkv.rearrange("(j p) c -> p j c", p=128))
            nc.sync.dma_start(out=wp_sb, in_=wproj.rearrange("(j p) c -> p j c", p=128))
            ident = const.tile([128, 128], BF16)
            make_identity(nc, ident)

            for w in range(NWIN):
                r0 = w * KW

                # --- load x^T window [512, 256] as 4 chunks of 128 channels
                xt_w = sb.tile([128, 4, KW], BF16)
                nc.gpsimd.dma_start(
                    out=xt_w,
                    in_=xt.rearrange("(j p) r -> p j r", p=128)[:, :, r0:r0 + KW],
                )

                # --- qk^T: channels of q,k (1024) chunked by 128 -> [128, 8, 256]
                qkT = sb.tile([128, 8, KW], BF16)
                for m in range(8):
                    acc = pqk.tile([128, KW], F32, tag="qk")
                    for j in range(4):
                        nc.tensor.matmul(
                            acc,
                            lhsT=wq_sb[:, j, m * 128:(m + 1) * 128],
                            rhs=xt_w[:, j, :],
                            start=(j == 0),
                            stop=(j == 3),
                        )
                    nc.vector.tensor_copy(qkT[:, m, :], acc)

                # --- v natural layout with ones column: [128rows, kc, head, 65]
                v_sb = sb.tile([128, 2, H, D + 1], BF16)
                nc.vector.memset(v_sb[:, :, :, D:D + 1], 1.0)
                for rc in range(2):
                    acc = pvf.tile([128, C], F32, tag="vf")
                    for j in range(4):
                        nc.tensor.matmul(
                            acc,
                            lhsT=xt_w[:, j, rc * 128:(rc + 1) * 128],
                            rhs=wq_sb[:, j, 2 * C:3 * C],
                            start=(j == 0),
                            stop=(j == 3),
                        )
                    # [128, 512] viewed as (H, D) -> strided into (H, 65) slots
                    nc.vector.tensor_copy(
                        v_sb[:, rc, :, 0:D],
                        acc.rearrange("p (h d) -> p h d", h=H),
                    )

                # --- attention per head ---------------------------------------
                attn = sb.tile([128, 2, C], BF16)  # natural [row, (head d)]
                for h in range(8):
                    poff = (h % 2) * 64
                    mq = h // 2          # q channels chunk
                    mk = 4 + h // 2      # k channels chunk
                    expT = eh.tile([128, 2, KW], BF16)
                    for kc in range(2):
                        sc = ps.tile([128, KW], F32, tag="s")
                        nc.tensor.matmul(
                            sc,
                            lhsT=qkT[poff:poff + 64, mk, kc * 128:(kc + 1) * 128],
                            rhs=qkT[poff:poff + 64, mq, :],
                            start=True,
                            stop=True,
                        )
                        # expT[k, q] = exp(scale * scoresT); no max-sub (|scores*scale| < ~8)
                        nc.scalar.activation(
                            expT[:, kc, :], sc,
                            mybir.ActivationFunctionType.Exp,
                            scale=SCALE,
                        )
                    for qc in range(2):
                        ov = pot.tile([128, D + 1], F32, tag="ot")
                        for kc in range(2):
                            nc.tensor.matmul(
                                ov,
                                lhsT=expT[:, kc, qc * 128:(qc + 1) * 128],
                                rhs=v_sb[:, kc, h, :],
                                start=(kc == 0),
                                stop=(kc == 1),
                            )
                        # normalize: out * (1/denom) ; denom is the ones-column result
                        rcp = eh.tile([128, 1], F32, tag="rcp")
                        nc.vector.reciprocal(rcp, ov[:, D:D + 1])
                        nc.vector.tensor_scalar(
                            attn[:, qc, h * D:(h + 1) * D],
                            ov[:, 0:D],
                            rcp,
                            None,
                            op0=mybir.AluOpType.mult,
                        )

                # --- transpose attention output for the projection -----------
                attnT = sb.tile([128, 4, KW], BF16)
                for rc in range(2):
                    for cc in range(4):
                        tp = pot.tile([128, 128], BF16, tag="ot")
                        nc.tensor.transpose(
                            tp, attn[:, rc, cc * 128:(cc + 1) * 128], ident
                        )
                        nc.vector.tensor_copy(attnT[:, cc, rc * 128:(rc + 1) * 128], tp)

                # --- projection + store --------------------------------------
                for rc in range(2):
                    acc = pvf.tile([128, C], F32, tag="vf")
                    for cc in range(4):
                        nc.tensor.matmul(
                            acc,
                            lhsT=attnT[:, cc, rc * 128:(rc + 1) * 128],
                            rhs=wp_sb[:, cc, :],
                            start=(cc == 0),
                            stop=(cc == 3),
                        )
                    o_sb = outp.tile([128, C], F32)
                    nc.vector.tensor_copy(o_sb, acc)
                    nc.sync.dma_start(
                        out=out[r0 + rc * 128:r0 + (rc + 1) * 128, :], in_=o_sb
                    )
    nc.finalize()
    return nc


_NC_CACHE = None


def _get_nc():
    global _NC_CACHE
    if _NC_CACHE is None:
        _NC_CACHE = build_nc()
    return _NC_CACHE


def _prep_in_maps(feat, order, Wqkv, Wproj):
    xs = np.asarray(feat, dtype=np.float32)[np.asarray(order)]
    wq = np.asarray(Wqkv, dtype=np.float32).astype(ml_dtypes.bfloat16)
    wp = np.asarray(Wproj, dtype=np.float32).astype(ml_dtypes.bfloat16)
    in_maps = []
    for m in range(NCORES):
        shard = xs[m * ROWS:(m + 1) * ROWS]
        xt = np.ascontiguousarray(shard.T).astype(ml_dtypes.bfloat16)
        in_maps.append({"xt": xt, "wqkv": wq, "wproj": wp})
    return in_maps


def kernel(feat, order, inverse, Wqkv, bqkv, Wproj, bproj, _trace=False):
    nc = _get_nc()
    in_maps = _prep_in_maps(feat, order, Wqkv, Wproj)
    res = run_bass_kernel_spmd(nc, in_maps, core_ids=list(range(NCORES)), trace=_trace)
    serial = np.concatenate([r["out"] for r in res.results], axis=0)
    final = serial[np.asarray(inverse)]
    # biases (host-side, exact): v-bias rides through softmax (rows sum to 1)
    # as + bv @ Wproj ; k-bias cancels in softmax ; q-bias is zero by spec.
    total_bias = (
        np.asarray(bqkv, dtype=np.float32)[2 * C:3 * C] @ np.asarray(Wproj, dtype=np.float32)
        + np.asarray(bproj, dtype=np.float32)
    )
    out = final + total_bias[None, :]
    if _trace:
        return out.astype(np.float32), res
    return out.astype(np.float32)


# revision 11
# speedup vs baseline: 1.2664x; 1.2664x over previous
"""Windowed (patch) attention kernel for 8 Trainium2 NeuronCores.

Problem: serialized point-cloud attention.
  qkv = feat @ Wqkv + bqkv ; qkv = qkv[order] -> windows of 256 rows
  per-window, per-head softmax attention ; out = attn_out[inverse] @ Wproj + bproj

Distribution strategy (per sharding hint): the permutation `order` is applied
host-side while sharding, so each core receives its 32 windows' rows already
gathered and channel-major (transposed).  All FLOPs (QKV proj, attention,
output proj) run on-device in bf16 with f32 PSUM accumulation.  `inverse`
scatter + bias adds are applied host-side (exact; row permutation commutes
with the row-wise projection, softmax is shift-invariant so the k-bias
cancels and the v-bias contributes bv @ Wproj to every row).
"""

import numpy as np
import ml_dtypes

import concourse.bass as bass
import concourse.mybir as mybir
from concourse import bacc
from concourse.tile import TileContext
from concourse.masks import make_identity
from concourse.bass_utils import run_bass_kernel_spmd

N = 65536
C = 512
H = 8
KW = 256          # window size
SCALE = 0.125
NCORES = 8
ROWS = N // NCORES        # 8192 rows per core
NWIN = ROWS // KW         # 32 windows per core
D = C // H                # 64 head dim

BF16 = mybir.dt.bfloat16
F32 = mybir.dt.float32


def build_nc():
    nc = bacc.Bacc("TRN2", target_bir_lowering=False, debug=False, num_devices=NCORES)

    xt = nc.declare_dram_parameter("xt", [C, ROWS], BF16, isOutput=False)
    wqkv = nc.declare_dram_parameter("wqkv", [C, 3 * C], BF16, isOutput=False)
    wproj = nc.declare_dram_parameter("wproj", [C, C], BF16, isOutput=False)
    out = nc.declare_dram_parameter("out", [ROWS, C], F32, isOutput=True)

    with TileContext(nc) as tc:
        with (
            tc.tile_pool(name="const", bufs=1) as const,
            tc.tile_pool(name="sb", bufs=2) as sb,
            tc.tile_pool(name="outp", bufs=3) as outp,
            tc.tile_pool(name="eh", bufs=3) as eh,
            tc.tile_pool(name="pqk", bufs=2, space="PSUM") as pqk,
            tc.tile_pool(name="ps", bufs=2, space="PSUM") as ps,
            tc.tile_pool(name="pot", bufs=2, space="PSUM") as pot,
            tc.tile_pool(name="pvf", bufs=2, space="PSUM") as pvf,
        ):
            # --- persistent tiles: weights + identity -----------------------
            wq_sb = const.tile([128, 4, 3 * C], BF16)
            wp_sb = const.tile([128, 4, C], BF16)
            nc.sync.dma_start(out=wq_sb, in_=wqkv.rearrange("(j p) c -> p j c", p=128))
            nc.sync.dma_start(out=wp_sb, in_=wproj.rearrange("(j p) c -> p j c", p=128))
            ident = const.tile([128, 128], BF16)
            make_identity(nc, ident)

            for w in range(NWIN):
                r0 = w * KW

                # --- load x^T window [512, 256] as 4 chunks of 128 channels
                xt_w = sb.tile([128, 4, KW], BF16)
                nc.gpsimd.dma_start(
                    out=xt_w,
                    in_=xt.rearrange("(j p) r -> p j r", p=128)[:, :, r0:r0 + KW],
                )

                # --- qk^T: channels of q,k (1024) chunked by 128 -> [128, 8, 256]
                qkT = sb.tile([128, 8, KW], BF16)
                for mp in range(4):  # pairs of channel chunks -> one PSUM bank
                    acc = pqk.tile([128, 2, KW], F32, tag="qk")
                    for half in range(2):
                        m = mp * 2 + half
                        for j in range(4):
                            nc.tensor.matmul(
                                acc[:, half, :],
                                lhsT=wq_sb[:, j, m * 128:(m + 1) * 128],
                                rhs=xt_w[:, j, :],
                                start=(j == 0),
                                stop=(j == 3),
                            )
                    nc.vector.tensor_copy(qkT[:, mp * 2:mp * 2 + 2, :], acc)

                # --- v natural layout with ones column: [128rows, kc, head, 65]
                v_sb = sb.tile([128, 2, H, D + 1], BF16)
                nc.vector.memset(v_sb[:, :, :, D:D + 1], 1.0)
                for rc in range(2):
                    acc = pvf.tile([128, C], F32, tag="vf")
                    for j in range(4):
                        nc.tensor.matmul(
                            acc,
                            lhsT=xt_w[:, j, rc * 128:(rc + 1) * 128],
                            rhs=wq_sb[:, j, 2 * C:3 * C],
                            start=(j == 0),
                            stop=(j == 3),
                        )
                    # [128, 512] viewed as (H, D) -> strided into (H, 65) slots
                    nc.vector.tensor_copy(
                        v_sb[:, rc, :, 0:D],
                        acc.rearrange("p (h d) -> p h d", h=H),
                    )

                # --- attention per head ---------------------------------------
                attn = sb.tile([128, 2, C], BF16)  # natural [row, (head d)]
                for h in range(8):
                    poff = (h % 2) * 64
                    mq = h // 2          # q channels chunk
                    mk = 4 + h // 2      # k channels chunk
                    expT = eh.tile([128, 2, KW], BF16)
                    sc = ps.tile([128, 2, KW], F32, tag="s")
                    for kc in range(2):
                        nc.tensor.matmul(
                            sc[:, kc, :],
                            lhsT=qkT[poff:poff + 64, mk, kc * 128:(kc + 1) * 128],
                            rhs=qkT[poff:poff + 64, mq, :],
                            start=True,
                            stop=True,
                        )
                    # expT[k, q] = exp(scale * scoresT); no max-sub (|scores*scale| < ~8)
                    nc.scalar.activation(
                        expT, sc,
                        mybir.ActivationFunctionType.Exp,
                        scale=SCALE,
                    )
                    ov = pot.tile([128, 2, D + 1], F32, tag="ot")
                    for qc in range(2):
                        for kc in range(2):
                            nc.tensor.matmul(
                                ov[:, qc, :],
                                lhsT=expT[:, kc, qc * 128:(qc + 1) * 128],
                                rhs=v_sb[:, kc, h, :],
                                start=(kc == 0),
                                stop=(kc == 1),
                            )
                    # normalize: out * (1/denom) ; denom is the ones-column result
                    rcp = eh.tile([128, 2, 1], F32, tag="rcp")
                    nc.vector.reciprocal(rcp, ov[:, :, D:D + 1])
                    nc.vector.tensor_mul(
                        attn[:, :, h * D:(h + 1) * D],
                        ov[:, :, 0:D],
                        rcp.broadcast_to([128, 2, D]),
                    )

                # --- transpose attention output for the projection -----------
                attnT = sb.tile([128, 4, KW], BF16)
                for rc in range(2):
                    tp = pot.tile([128, 4, 128], BF16, tag="ot")
                    for cc in range(4):
                        nc.tensor.transpose(
                            tp[:, cc, :], attn[:, rc, cc * 128:(cc + 1) * 128], ident
                        )
                    nc.vector.tensor_copy(attnT[:, :, rc * 128:(rc + 1) * 128], tp)

                # --- projection + store --------------------------------------
                for rc in range(2):
                    acc = pvf.tile([128, C], F32, tag="vf")
                    for cc in range(4):
                        nc.tensor.matmul(
                            acc,
                            lhsT=attnT[:, cc, rc * 128:(rc + 1) * 128],
                            rhs=wp_sb[:, cc, :],
                            start=(cc == 0),
                            stop=(cc == 3),
                        )
                    o_sb = outp.tile([128, C], F32)
                    nc.vector.tensor_copy(o_sb, acc)
                    nc.sync.dma_start(
                        out=out[r0 + rc * 128:r0 + (rc + 1) * 128, :], in_=o_sb
                    )
    nc.finalize()
    return nc


_NC_CACHE = None


def _get_nc():
    global _NC_CACHE
    if _NC_CACHE is None:
        _NC_CACHE = build_nc()
    return _NC_CACHE


def _prep_in_maps(feat, order, Wqkv, Wproj):
    xs = np.asarray(feat, dtype=np.float32)[np.asarray(order)]
    wq = np.asarray(Wqkv, dtype=np.float32).astype(ml_dtypes.bfloat16)
    wp = np.asarray(Wproj, dtype=np.float32).astype(ml_dtypes.bfloat16)
    in_maps = []
    for m in range(NCORES):
        shard = xs[m * ROWS:(m + 1) * ROWS]
        xt = np.ascontiguousarray(shard.T).astype(ml_dtypes.bfloat16)
        in_maps.append({"xt": xt, "wqkv": wq, "wproj": wp})
    return in_maps


def kernel(feat, order, inverse, Wqkv, bqkv, Wproj, bproj, _trace=False):
    nc = _get_nc()
    in_maps = _prep_in_maps(feat, order, Wqkv, Wproj)
    res = run_bass_kernel_spmd(nc, in_maps, core_ids=list(range(NCORES)), trace=_trace)
    serial = np.concatenate([r["out"] for r in res.results], axis=0)
    final = serial[np.asarray(inverse)]
    # biases (host-side, exact): v-bias rides through softmax (rows sum to 1)
    # as + bv @ Wproj ; k-bias cancels in softmax ; q-bias is zero by spec.
    total_bias = (
        np.asarray(bqkv, dtype=np.float32)[2 * C:3 * C] @ np.asarray(Wproj, dtype=np.float32)
        + np.asarray(bproj, dtype=np.float32)
    )
    out = final + total_bias[None, :]
    if _trace:
        return out.astype(np.float32), res
    return out.astype(np.float32)


# revision 13
# speedup vs baseline: 1.6493x; 1.3024x over previous
"""Windowed (patch) attention kernel for 8 Trainium2 NeuronCores.

Problem: serialized point-cloud attention.
  qkv = feat @ Wqkv + bqkv ; qkv = qkv[order] -> windows of 256 rows
  per-window, per-head softmax attention ; out = attn_out[inverse] @ Wproj + bproj

Distribution strategy (per sharding hint): the permutation `order` is applied
host-side while sharding, so each core receives its 32 windows' rows already
gathered and channel-major (transposed).  All FLOPs (QKV proj, attention,
output proj) run on-device in bf16 with f32 PSUM accumulation.  `inverse`
scatter + bias adds are applied host-side (exact; row permutation commutes
with the row-wise projection, softmax is shift-invariant so the k-bias
cancels and the v-bias contributes bv @ Wproj to every row).
"""

import numpy as np
import ml_dtypes

import concourse.bass as bass
import concourse.mybir as mybir
from concourse import bacc
from concourse.tile import TileContext
from concourse.masks import make_identity
from concourse.bass_utils import run_bass_kernel_spmd

N = 65536
C = 512
H = 8
KW = 256          # window size
SCALE = 0.125
NCORES = 8
ROWS = N // NCORES        # 8192 rows per core
NWIN = ROWS // KW         # 32 windows per core
D = C // H                # 64 head dim

BF16 = mybir.dt.bfloat16
F32 = mybir.dt.float32


def build_nc():
    nc = bacc.Bacc("TRN2", target_bir_lowering=False, debug=False, num_devices=NCORES)

    xt = nc.declare_dram_parameter("xt", [C, ROWS], BF16, isOutput=False)
    wqkv = nc.declare_dram_parameter("wqkv", [C, 3 * C], BF16, isOutput=False)
    wproj = nc.declare_dram_parameter("wproj", [C, C], BF16, isOutput=False)
    out = nc.declare_dram_parameter("out", [ROWS, C], F32, isOutput=True)

    with TileContext(nc) as tc:
        with (
            tc.tile_pool(name="const", bufs=1) as const,
            tc.tile_pool(name="sb", bufs=2) as sb,
            tc.tile_pool(name="outp", bufs=3) as outp,
            tc.tile_pool(name="eh", bufs=3) as eh,
            tc.tile_pool(name="pqk", bufs=2, space="PSUM") as pqk,
            tc.tile_pool(name="ps", bufs=2, space="PSUM") as ps,
            tc.tile_pool(name="pot", bufs=2, space="PSUM") as pot,
            tc.tile_pool(name="pvf", bufs=2, space="PSUM") as pvf,
        ):
            # --- persistent tiles: weights + identity -----------------------
            wq_sb = const.tile([128, 4, 3 * C], BF16)
            wp_sb = const.tile([128, 4, C], BF16)
            nc.sync.dma_start(out=wq_sb, in_=wqkv.rearrange("(j p) c -> p j c", p=128))
            nc.sync.dma_start(out=wp_sb, in_=wproj.rearrange("(j p) c -> p j c", p=128))
            ident = const.tile([128, 128], BF16)
            make_identity(nc, ident)

            GW = 2 * KW  # 2 windows per group: N=512 matmuls for qkv
            for wg in range(NWIN // 2):
                g0 = wg * GW

                # --- load x^T for the group [512, 512]
                xt_g = sb.tile([128, 4, GW], BF16)
                nc.gpsimd.dma_start(
                    out=xt_g,
                    in_=xt.rearrange("(j p) r -> p j r", p=128)[:, :, g0:g0 + GW],
                )

                # --- qk^T: q,k channels (1024) chunked by 128 -> [128, 8, 512]
                qkT = sb.tile([128, 8, GW], BF16)
                for m in range(8):
                    acc = pqk.tile([128, GW], F32, tag="qk")
                    for j in range(4):
                        nc.tensor.matmul(
                            acc,
                            lhsT=wq_sb[:, j, m * 128:(m + 1) * 128],
                            rhs=xt_g[:, j, :],
                            start=(j == 0),
                            stop=(j == 3),
                        )
                    nc.vector.tensor_copy(qkT[:, m, :], acc)

                # --- v natural layout with ones column: [128rows, rc, head, 65]
                v_sb = sb.tile([128, 4, H, D + 1], BF16)
                nc.vector.memset(v_sb[:, :, :, D:D + 1], 1.0)
                for rc in range(4):
                    acc = pvf.tile([128, C], F32, tag="vf")
                    for j in range(4):
                        nc.tensor.matmul(
                            acc,
                            lhsT=xt_g[:, j, rc * 128:(rc + 1) * 128],
                            rhs=wq_sb[:, j, 2 * C:3 * C],
                            start=(j == 0),
                            stop=(j == 3),
                        )
                    # [128, 512] viewed as (H, D) -> strided into (H, 65) slots
                    nc.vector.tensor_copy(
                        v_sb[:, rc, :, 0:D],
                        acc.rearrange("p (h d) -> p h d", h=H),
                    )

                for wi in range(2):
                    r0 = g0 + wi * KW
                    qoff = wi * KW
                    # --- attention, head pairs (even head rows 0-63, odd 64-127
                    # of the same qkT chunk -> concurrent PE sub-array use)
                    attn = sb.tile([128, 2, C], BF16)  # natural [row, (head d)]
                    for hp in range(4):
                        scs = []
                        exps = []
                        for hh in range(2):
                            scs.append(ps.tile([128, 2, KW], F32, tag="s", name=f"sc{hh}"))
                            exps.append(eh.tile([128, 2, KW], BF16, tag="expT", name=f"expT{hh}"))
                        for kc in range(2):
                            for hh in range(2):
                                poff = hh * 64
                                nc.tensor.matmul(
                                    scs[hh][:, kc, :],
                                    lhsT=qkT[poff:poff + 64, 4 + hp,
                                             qoff + kc * 128:qoff + (kc + 1) * 128],
                                    rhs=qkT[poff:poff + 64, hp, qoff:qoff + KW],
                                    start=True,
                                    stop=True,
                                )
                        for hh in range(2):
                            # expT[k,q] = exp(scale*scoresT); no max-sub
                            # (|scores*scale| bounded ~8 for these inputs)
                            nc.scalar.activation(
                                exps[hh], scs[hh],
                                mybir.ActivationFunctionType.Exp,
                                scale=SCALE,
                            )
                        for hh in range(2):
                            h = 2 * hp + hh
                            ov = pot.tile([128, 2, D + 1], F32, tag="ot")
                            for qc in range(2):
                                for kc in range(2):
                                    nc.tensor.matmul(
                                        ov[:, qc, :],
                                        lhsT=exps[hh][:, kc, qc * 128:(qc + 1) * 128],
                                        rhs=v_sb[:, wi * 2 + kc, h, :],
                                        start=(kc == 0),
                                        stop=(kc == 1),
                                    )
                            # normalize by the ones-column result
                            rcp = eh.tile([128, 2, 1], F32, tag="rcp")
                            nc.vector.reciprocal(rcp, ov[:, :, D:D + 1])
                            nc.vector.tensor_mul(
                                attn[:, :, h * D:(h + 1) * D],
                                ov[:, :, 0:D],
                                rcp.broadcast_to([128, 2, D]),
                            )

                    # --- transpose attention output for the projection -------
                    attnT = sb.tile([128, 4, KW], BF16)
                    for rc in range(2):
                        tp = pot.tile([128, 4, 128], BF16, tag="ot")
                        for cc in range(4):
                            nc.tensor.transpose(
                                tp[:, cc, :], attn[:, rc, cc * 128:(cc + 1) * 128],
                                ident,
                            )
                        nc.vector.tensor_copy(attnT[:, :, rc * 128:(rc + 1) * 128], tp)

                    # --- projection + store ----------------------------------
                    for rc in range(2):
                        acc = pvf.tile([128, C], F32, tag="vf")
                        for cc in range(4):
                            nc.tensor.matmul(
                                acc,
                                lhsT=attnT[:, cc, rc * 128:(rc + 1) * 128],
                                rhs=wp_sb[:, cc, :],
                                start=(cc == 0),
                                stop=(cc == 3),
                            )
                        o_sb = outp.tile([128, C], F32)
                        nc.vector.tensor_copy(o_sb, acc)
                        nc.sync.dma_start(
                            out=out[r0 + rc * 128:r0 + (rc + 1) * 128, :], in_=o_sb
                        )
    nc.finalize()
    return nc


_NC_CACHE = None


def _get_nc():
    global _NC_CACHE
    if _NC_CACHE is None:
        _NC_CACHE = build_nc()
    return _NC_CACHE


def _prep_in_maps(feat, order, Wqkv, Wproj):
    xs = np.asarray(feat, dtype=np.float32)[np.asarray(order)]
    wq = np.asarray(Wqkv, dtype=np.float32).astype(ml_dtypes.bfloat16)
    wp = np.asarray(Wproj, dtype=np.float32).astype(ml_dtypes.bfloat16)
    in_maps = []
    for m in range(NCORES):
        shard = xs[m * ROWS:(m + 1) * ROWS]
        xt = np.ascontiguousarray(shard.T).astype(ml_dtypes.bfloat16)
        in_maps.append({"xt": xt, "wqkv": wq, "wproj": wp})
    return in_maps


def kernel(feat, order, inverse, Wqkv, bqkv, Wproj, bproj, _trace=False):
    nc = _get_nc()
    in_maps = _prep_in_maps(feat, order, Wqkv, Wproj)
    res = run_bass_kernel_spmd(nc, in_maps, core_ids=list(range(NCORES)), trace=_trace)
    serial = np.concatenate([r["out"] for r in res.results], axis=0)
    final = serial[np.asarray(inverse)]
    # biases (host-side, exact): v-bias rides through softmax (rows sum to 1)
    # as + bv @ Wproj ; k-bias cancels in softmax ; q-bias is zero by spec.
    total_bias = (
        np.asarray(bqkv, dtype=np.float32)[2 * C:3 * C] @ np.asarray(Wproj, dtype=np.float32)
        + np.asarray(bproj, dtype=np.float32)
    )
    out = final + total_bias[None, :]
    if _trace:
        return out.astype(np.float32), res
    return out.astype(np.float32)


# revision 14
# speedup vs baseline: 1.6592x; 1.0060x over previous
"""Windowed (patch) attention kernel for 8 Trainium2 NeuronCores.

Problem: serialized point-cloud attention.
  qkv = feat @ Wqkv + bqkv ; qkv = qkv[order] -> windows of 256 rows
  per-window, per-head softmax attention ; out = attn_out[inverse] @ Wproj + bproj

Distribution strategy (per sharding hint): the permutation `order` is applied
host-side while sharding, so each core receives its 32 windows' rows already
gathered and channel-major (transposed).  All FLOPs (QKV proj, attention,
output proj) run on-device in bf16 with f32 PSUM accumulation.  `inverse`
scatter + bias adds are applied host-side (exact; row permutation commutes
with the row-wise projection, softmax is shift-invariant so the k-bias
cancels and the v-bias contributes bv @ Wproj to every row).
"""

import numpy as np
import ml_dtypes

import concourse.bass as bass
import concourse.mybir as mybir
from concourse import bacc
from concourse.tile import TileContext
from concourse.masks import make_identity
from concourse.bass_utils import run_bass_kernel_spmd

N = 65536
C = 512
H = 8
KW = 256          # window size
SCALE = 0.125
NCORES = 8
ROWS = N // NCORES        # 8192 rows per core
NWIN = ROWS // KW         # 32 windows per core
D = C // H                # 64 head dim

BF16 = mybir.dt.bfloat16
F32 = mybir.dt.float32


def build_nc():
    nc = bacc.Bacc("TRN2", target_bir_lowering=False, debug=False, num_devices=NCORES)

    xt = nc.declare_dram_parameter("xt", [C, ROWS], BF16, isOutput=False)
    wqkv = nc.declare_dram_parameter("wqkv", [C, 3 * C], BF16, isOutput=False)
    wproj = nc.declare_dram_parameter("wproj", [C, C], BF16, isOutput=False)
    out = nc.declare_dram_parameter("out", [ROWS, C], F32, isOutput=True)

    with TileContext(nc) as tc:
        with (
            tc.tile_pool(name="const", bufs=1) as const,
            tc.tile_pool(name="sb", bufs=2) as sb,
            tc.tile_pool(name="outp", bufs=3) as outp,
            tc.tile_pool(name="eh", bufs=3) as eh,
            tc.tile_pool(name="pqk", bufs=2, space="PSUM") as pqk,
            tc.tile_pool(name="ps", bufs=2, space="PSUM") as ps,
            tc.tile_pool(name="pot", bufs=2, space="PSUM") as pot,
            tc.tile_pool(name="pvf", bufs=2, space="PSUM") as pvf,
        ):
            # --- persistent tiles: weights + identity -----------------------
            wq_sb = const.tile([128, 4, 3 * C], BF16)
            wp_sb = const.tile([128, 4, C], BF16)
            nc.sync.dma_start(out=wq_sb, in_=wqkv.rearrange("(j p) c -> p j c", p=128))
            nc.sync.dma_start(out=wp_sb, in_=wproj.rearrange("(j p) c -> p j c", p=128))
            ident = const.tile([128, 128], BF16)
            make_identity(nc, ident)

            GW = 2 * KW  # 2 windows per group: N=512 matmuls for qkv
            for wg in range(NWIN // 2):
                g0 = wg * GW

                # --- load x^T for the group [512, 512]
                xt_g = sb.tile([128, 4, GW], BF16)
                nc.gpsimd.dma_start(
                    out=xt_g,
                    in_=xt.rearrange("(j p) r -> p j r", p=128)[:, :, g0:g0 + GW],
                )

                # --- qk^T: q,k channels (1024) chunked by 128 -> [128, 8, 512]
                qkT = sb.tile([128, 8, GW], BF16)
                for m in range(8):
                    acc = pqk.tile([128, GW], F32, tag="qk")
                    for j in range(4):
                        nc.tensor.matmul(
                            acc,
                            lhsT=wq_sb[:, j, m * 128:(m + 1) * 128],
                            rhs=xt_g[:, j, :],
                            start=(j == 0),
                            stop=(j == 3),
                        )
                    nc.vector.tensor_copy(qkT[:, m, :], acc)

                # --- v natural layout with ones column: [128rows, rc, head, 65]
                v_sb = sb.tile([128, 4, H, D + 1], BF16)
                nc.vector.memset(v_sb[:, :, :, D:D + 1], 1.0)
                for rc in range(4):
                    acc = pvf.tile([128, C], F32, tag="vf")
                    for j in range(4):
                        nc.tensor.matmul(
                            acc,
                            lhsT=xt_g[:, j, rc * 128:(rc + 1) * 128],
                            rhs=wq_sb[:, j, 2 * C:3 * C],
                            start=(j == 0),
                            stop=(j == 3),
                        )
                    # [128, 512] viewed as (H, D) -> strided into (H, 65) slots
                    nc.vector.tensor_copy(
                        v_sb[:, rc, :, 0:D],
                        acc.rearrange("p (h d) -> p h d", h=H),
                    )

                for wi in range(2):
                    r0 = g0 + wi * KW
                    qoff = wi * KW
                    # --- attention, head pairs (even head rows 0-63, odd 64-127
                    # of the same qkT chunk -> concurrent PE sub-array use)
                    attn = sb.tile([128, 2, C], BF16)  # natural [row, (head d)]
                    for hp in range(4):
                        scs = []
                        exps = []
                        for hh in range(2):
                            scs.append(ps.tile([128, 2, KW], F32, tag="s", name=f"sc{hh}"))
                            exps.append(eh.tile([128, 2, KW], BF16, tag="expT", name=f"expT{hh}"))
                        for kc in range(2):
                            for hh in range(2):
                                poff = hh * 64
                                nc.tensor.matmul(
                                    scs[hh][:, kc, :],
                                    lhsT=qkT[poff:poff + 64, 4 + hp,
                                             qoff + kc * 128:qoff + (kc + 1) * 128],
                                    rhs=qkT[poff:poff + 64, hp, qoff:qoff + KW],
                                    start=True,
                                    stop=True,
                                )
                        for hh in range(2):
                            # expT[k,q] = exp(scale*scoresT); no max-sub
                            # (|scores*scale| bounded ~8 for these inputs)
                            nc.scalar.activation(
                                exps[hh], scs[hh],
                                mybir.ActivationFunctionType.Exp,
                                scale=SCALE,
                            )
                        # both heads' attn@V into one PSUM bank: (qc, hh, 65)
                        ov = pot.tile([128, 2, 2, D + 1], F32, tag="ot")
                        for hh in range(2):
                            h = 2 * hp + hh
                            for qc in range(2):
                                for kc in range(2):
                                    nc.tensor.matmul(
                                        ov[:, qc, hh, :],
                                        lhsT=exps[hh][:, kc, qc * 128:(qc + 1) * 128],
                                        rhs=v_sb[:, wi * 2 + kc, h, :],
                                        start=(kc == 0),
                                        stop=(kc == 1),
                                    )
                        # normalize by the ones-column result (both heads at once)
                        rcp = eh.tile([128, 2, 2, 1], F32, tag="rcp")
                        nc.vector.reciprocal(rcp, ov[:, :, :, D:D + 1])
                        nc.vector.tensor_mul(
                            attn[:, :, 2 * hp * D:(2 * hp + 2) * D].rearrange(
                                "p q (e d) -> p q e d", e=2
                            ),
                            ov[:, :, :, 0:D],
                            rcp.broadcast_to([128, 2, 2, D]),
                        )

                    # --- transpose attention output for the projection -------
                    # DMA xbar transpose (bf16): [row, ch] -> [ch, row]
                    attnT = sb.tile([128, 4, KW], BF16)
                    for qc in range(2):
                        nc.sync.dma_start_transpose(
                            out=attnT[:, :, qc * 128:(qc + 1) * 128],
                            in_=attn[:, qc, :],
                        )

                    # --- projection + store ----------------------------------
                    for rc in range(2):
                        acc = pvf.tile([128, C], F32, tag="vf")
                        for cc in range(4):
                            nc.tensor.matmul(
                                acc,
                                lhsT=attnT[:, cc, rc * 128:(rc + 1) * 128],
                                rhs=wp_sb[:, cc, :],
                                start=(cc == 0),
                                stop=(cc == 3),
                            )
                        o_sb = outp.tile([128, C], F32)
                        nc.vector.tensor_copy(o_sb, acc)
                        nc.sync.dma_start(
                            out=out[r0 + rc * 128:r0 + (rc + 1) * 128, :], in_=o_sb
                        )
    nc.finalize()
    return nc


_NC_CACHE = None


def _get_nc():
    global _NC_CACHE
    if _NC_CACHE is None:
        _NC_CACHE = build_nc()
    return _NC_CACHE


def _prep_in_maps(feat, order, Wqkv, Wproj):
    xs = np.asarray(feat, dtype=np.float32)[np.asarray(order)]
    wq = np.asarray(Wqkv, dtype=np.float32).astype(ml_dtypes.bfloat16)
    wp = np.asarray(Wproj, dtype=np.float32).astype(ml_dtypes.bfloat16)
    in_maps = []
    for m in range(NCORES):
        shard = xs[m * ROWS:(m + 1) * ROWS]
        xt = np.ascontiguousarray(shard.T).astype(ml_dtypes.bfloat16)
        in_maps.append({"xt": xt, "wqkv": wq, "wproj": wp})
    return in_maps


def kernel(feat, order, inverse, Wqkv, bqkv, Wproj, bproj, _trace=False):
    nc = _get_nc()
    in_maps = _prep_in_maps(feat, order, Wqkv, Wproj)
    res = run_bass_kernel_spmd(nc, in_maps, core_ids=list(range(NCORES)), trace=_trace)
    serial = np.concatenate([r["out"] for r in res.results], axis=0)
    final = serial[np.asarray(inverse)]
    # biases (host-side, exact): v-bias rides through softmax (rows sum to 1)
    # as + bv @ Wproj ; k-bias cancels in softmax ; q-bias is zero by spec.
    total_bias = (
        np.asarray(bqkv, dtype=np.float32)[2 * C:3 * C] @ np.asarray(Wproj, dtype=np.float32)
        + np.asarray(bproj, dtype=np.float32)
    )
    out = final + total_bias[None, :]
    if _trace:
        return out.astype(np.float32), res
    return out.astype(np.float32)


# revision 16
# speedup vs baseline: 1.6641x; 1.0029x over previous
"""Windowed (patch) attention kernel for 8 Trainium2 NeuronCores.

Problem: serialized point-cloud attention.
  qkv = feat @ Wqkv + bqkv ; qkv = qkv[order] -> windows of 256 rows
  per-window, per-head softmax attention ; out = attn_out[inverse] @ Wproj + bproj

Distribution strategy (per sharding hint): the permutation `order` is applied
host-side while sharding, so each core receives its 32 windows' rows already
gathered and channel-major (transposed).  All FLOPs (QKV proj, attention,
output proj) run on-device in bf16 with f32 PSUM accumulation.  `inverse`
scatter + bias adds are applied host-side (exact; row permutation commutes
with the row-wise projection, softmax is shift-invariant so the k-bias
cancels and the v-bias contributes bv @ Wproj to every row).
"""

import numpy as np
import ml_dtypes

import concourse.bass as bass
import concourse.mybir as mybir
from concourse import bacc
from concourse.tile import TileContext
from concourse.masks import make_identity
from concourse.bass_utils import run_bass_kernel_spmd

N = 65536
C = 512
H = 8
KW = 256          # window size
SCALE = 0.125
NCORES = 8
ROWS = N // NCORES        # 8192 rows per core
NWIN = ROWS // KW         # 32 windows per core
D = C // H                # 64 head dim

BF16 = mybir.dt.bfloat16
F32 = mybir.dt.float32


def build_nc():
    nc = bacc.Bacc("TRN2", target_bir_lowering=False, debug=False, num_devices=NCORES)

    xt = nc.declare_dram_parameter("xt", [C, ROWS], BF16, isOutput=False)
    wqkv = nc.declare_dram_parameter("wqkv", [C, 3 * C], BF16, isOutput=False)
    wproj = nc.declare_dram_parameter("wproj", [C, C], BF16, isOutput=False)
    out = nc.declare_dram_parameter("out", [ROWS, C], F32, isOutput=True)

    with TileContext(nc) as tc:
        with (
            tc.tile_pool(name="const", bufs=1) as const,
            tc.tile_pool(name="sb", bufs=3) as sb,
            tc.tile_pool(name="outp", bufs=3) as outp,
            tc.tile_pool(name="eh", bufs=3) as eh,
            tc.tile_pool(name="pqk", bufs=2, space="PSUM") as pqk,
            tc.tile_pool(name="ps", bufs=2, space="PSUM") as ps,
            tc.tile_pool(name="pot", bufs=2, space="PSUM") as pot,
            tc.tile_pool(name="pvf", bufs=2, space="PSUM") as pvf,
        ):
            # --- persistent tiles: weights + identity -----------------------
            wq_sb = const.tile([128, 4, 3 * C], BF16)
            wp_sb = const.tile([128, 4, C], BF16)
            nc.sync.dma_start(out=wq_sb, in_=wqkv.rearrange("(j p) c -> p j c", p=128))
            nc.sync.dma_start(out=wp_sb, in_=wproj.rearrange("(j p) c -> p j c", p=128))
            ident = const.tile([128, 128], BF16)
            make_identity(nc, ident)

            GW = 2 * KW  # 2 windows per group: N=512 matmuls for qkv
            for wg in range(NWIN // 2):
                g0 = wg * GW

                # --- load x^T for the group [512, 512]
                xt_g = sb.tile([128, 4, GW], BF16)
                nc.gpsimd.dma_start(
                    out=xt_g,
                    in_=xt.rearrange("(j p) r -> p j r", p=128)[:, :, g0:g0 + GW],
                )

                # --- qk^T: q,k channels (1024) chunked by 128 -> [128, 8, 512]
                qkT = sb.tile([128, 8, GW], BF16)
                for m in range(8):
                    acc = pqk.tile([128, GW], F32, tag="qk")
                    for j in range(4):
                        nc.tensor.matmul(
                            acc,
                            lhsT=wq_sb[:, j, m * 128:(m + 1) * 128],
                            rhs=xt_g[:, j, :],
                            start=(j == 0),
                            stop=(j == 3),
                        )
                    nc.vector.tensor_copy(qkT[:, m, :], acc)

                # --- v natural layout with ones column: [128rows, rc, head, 65]
                v_sb = sb.tile([128, 4, H, D + 1], BF16)
                nc.vector.memset(v_sb[:, :, :, D:D + 1], 1.0)
                for rc in range(4):
                    acc = pvf.tile([128, C], F32, tag="vf")
                    for j in range(4):
                        nc.tensor.matmul(
                            acc,
                            lhsT=xt_g[:, j, rc * 128:(rc + 1) * 128],
                            rhs=wq_sb[:, j, 2 * C:3 * C],
                            start=(j == 0),
                            stop=(j == 3),
                        )
                    # [128, 512] viewed as (H, D) -> strided into (H, 65) slots
                    nc.vector.tensor_copy(
                        v_sb[:, rc, :, 0:D],
                        acc.rearrange("p (h d) -> p h d", h=H),
                    )

                for wi in range(2):
                    r0 = g0 + wi * KW
                    qoff = wi * KW
                    # --- attention, head pairs (even head rows 0-63, odd 64-127
                    # of the same qkT chunk -> concurrent PE sub-array use)
                    attn = sb.tile([128, 2, C], BF16)  # natural [row, (head d)]
                    for hp in range(4):
                        scs = []
                        exps = []
                        for hh in range(2):
                            scs.append(ps.tile([128, 2, KW], F32, tag="s", name=f"sc{hh}"))
                            exps.append(eh.tile([128, 2, KW], BF16, tag="expT", name=f"expT{hh}"))
                        for kc in range(2):
                            for hh in range(2):
                                poff = hh * 64
                                nc.tensor.matmul(
                                    scs[hh][:, kc, :],
                                    lhsT=qkT[poff:poff + 64, 4 + hp,
                                             qoff + kc * 128:qoff + (kc + 1) * 128],
                                    rhs=qkT[poff:poff + 64, hp, qoff:qoff + KW],
                                    start=True,
                                    stop=True,
                                    tile_position=(poff, 0),
                                )
                        for hh in range(2):
                            # expT[k,q] = exp(scale*scoresT); no max-sub
                            # (|scores*scale| bounded ~8 for these inputs)
                            nc.scalar.activation(
                                exps[hh], scs[hh],
                                mybir.ActivationFunctionType.Exp,
                                scale=SCALE,
                            )
                        # both heads' attn@V into one PSUM bank: (qc, hh, 65)
                        ov = pot.tile([128, 2, 2, D + 1], F32, tag="ot")
                        for hh in range(2):
                            h = 2 * hp + hh
                            for qc in range(2):
                                for kc in range(2):
                                    nc.tensor.matmul(
                                        ov[:, qc, hh, :],
                                        lhsT=exps[hh][:, kc, qc * 128:(qc + 1) * 128],
                                        rhs=v_sb[:, wi * 2 + kc, h, :],
                                        start=(kc == 0),
                                        stop=(kc == 1),
                                    )
                        # normalize by the ones-column result (both heads at once)
                        rcp = eh.tile([128, 2, 2, 1], F32, tag="rcp")
                        nc.vector.reciprocal(rcp, ov[:, :, :, D:D + 1])
                        nc.vector.tensor_mul(
                            attn[:, :, 2 * hp * D:(2 * hp + 2) * D].rearrange(
                                "p q (e d) -> p q e d", e=2
                            ),
                            ov[:, :, :, 0:D],
                            rcp.broadcast_to([128, 2, 2, D]),
                        )

                    # --- transpose attention output for the projection -------
                    # DMA xbar transpose (bf16): [row, ch] -> [ch, row]
                    attnT = sb.tile([128, 4, KW], BF16)
                    for qc in range(2):
                        nc.sync.dma_start_transpose(
                            out=attnT[:, :, qc * 128:(qc + 1) * 128],
                            in_=attn[:, qc, :],
                        )

                    # --- projection + store ----------------------------------
                    for rc in range(2):
                        acc = pvf.tile([128, C], F32, tag="vf")
                        for cc in range(4):
                            nc.tensor.matmul(
                                acc,
                                lhsT=attnT[:, cc, rc * 128:(rc + 1) * 128],
                                rhs=wp_sb[:, cc, :],
                                start=(cc == 0),
                                stop=(cc == 3),
                            )
                        o_sb = outp.tile([128, C], F32)
                        nc.vector.tensor_copy(o_sb, acc)
                        nc.sync.dma_start(
                            out=out[r0 + rc * 128:r0 + (rc + 1) * 128, :], in_=o_sb
                        )
    nc.finalize()
    return nc


_NC_CACHE = None


def _get_nc():
    global _NC_CACHE
    if _NC_CACHE is None:
        _NC_CACHE = build_nc()
    return _NC_CACHE


def _prep_in_maps(feat, order, Wqkv, Wproj):
    xs = np.asarray(feat, dtype=np.float32)[np.asarray(order)]
    wq = np.asarray(Wqkv, dtype=np.float32).astype(ml_dtypes.bfloat16)
    wp = np.asarray(Wproj, dtype=np.float32).astype(ml_dtypes.bfloat16)
    in_maps = []
    for m in range(NCORES):
        shard = xs[m * ROWS:(m + 1) * ROWS]
        xt = np.ascontiguousarray(shard.T).astype(ml_dtypes.bfloat16)
        in_maps.append({"xt": xt, "wqkv": wq, "wproj": wp})
    return in_maps


def kernel(feat, order, inverse, Wqkv, bqkv, Wproj, bproj, _trace=False):
    nc = _get_nc()
    in_maps = _prep_in_maps(feat, order, Wqkv, Wproj)
    res = run_bass_kernel_spmd(nc, in_maps, core_ids=list(range(NCORES)), trace=_trace)
    serial = np.concatenate([r["out"] for r in res.results], axis=0)
    final = serial[np.asarray(inverse)]
    # biases (host-side, exact): v-bias rides through softmax (rows sum to 1)
    # as + bv @ Wproj ; k-bias cancels in softmax ; q-bias is zero by spec.
    total_bias = (
        np.asarray(bqkv, dtype=np.float32)[2 * C:3 * C] @ np.asarray(Wproj, dtype=np.float32)
        + np.asarray(bproj, dtype=np.float32)
    )
    out = final + total_bias[None, :]
    if _trace:
        return out.astype(np.float32), res
    return out.astype(np.float32)
